# revision 6
# baseline (speedup 1.0000x reference)
"""Trainium2 Bass kernel for nn_LlamaAttentionPNA_LM.

Sharding: 8 cores, 2 heads per core (tensor-parallel over heads).
Each core computes its 2 heads end-to-end plus a partial o_proj product
over the full output; the host sums the 8 partials (the "all-reduce").

Design (v2, rewritten for the TimelineSim cost model):
  - All matmuls run as float32r (exact fp32 in sim, 1 cycle/row when the
    output free dim is >= 256) or bf16/fp16 (1 cycle/row always).
  - Top-k per row uses 2-pass rounds (max8 + match_replace) on DVE; the
    per-row selection cutoff is the k_i-th entry of the sorted `vals`
    (one-hot mask + max-reduce), and the adjacency is a single
    tensor_scalar `g >= cutoff` pass (0.5x DVE mode).
  - Gather index lists come from a Pool-side pipeline: prefix-scan of the
    adjacency -> masked ranks -> local_scatter compaction (per-partition
    scatter), then replication into the 8 gpsimd core groups via fp16 PE
    matmuls with constant one-hot REP matrices + an Act-engine psum->int16
    convert with a +(S-1) sentinel bias (empty slots gather the NEG
    sentinel column S-1 of vTg).
  - Aggregation (sum/sumsq) via bf16 matmuls over per-block transposed
    adjacency; mean/var on DVE; max aggregator via gpsimd ap_gather of
    vTg + DVE segmented max-reduce.
  - Per-head GIN MLP and the o_proj partial in bf16/f32r; output DMAs go
    straight from PSUM to DRAM.
"""

import numpy as np
from contextlib import ExitStack

import concourse.bass as bass
from concourse import bacc
import concourse.mybir as mybir
import concourse.tile as tile
from concourse.bass_utils import run_bass_kernel_spmd
from concourse.masks import make_identity

F32 = mybir.dt.float32
F32R = mybir.dt.float32r
BF16 = mybir.dt.bfloat16
FP16 = mybir.dt.float16
U16 = mybir.dt.uint16
U8 = mybir.dt.uint8
I16 = mybir.dt.int16

H, D, HID, S = 16, 64, 1024, 1024
MULT = 2
FRAC, THR, BASE = 0.1, 0.2, 10000.0
NEG = -1e30
DELTA = 1e-8
NCHUNK = S // 128
NCORES = 8
AF = mybir.ActivationFunctionType
ALU = mybir.AluOpType


def _k_vec():
    # Must match jnp.maximum(1, ceil(f32(0.1) * arange(S, f32))), k[0]=0.
    k = np.ceil(np.float32(FRAC) * np.arange(S, dtype=np.float32)).astype(np.int64)
    k = np.maximum(k, 1)
    k[0] = 0
    return k


KV = _k_vec()
KMAXC = [int(KV[128 * (c + 1) - 1]) for c in range(NCHUNK)]      # max k per chunk
RC = [(km + 7) // 8 for km in KMAXC]                             # max8 rounds
KPAD = [(km + 3) // 4 * 4 for km in KMAXC]   # gather pad width (4-elem aligned)
KHW = [8 * r for r in RC]
KHOFF = np.concatenate([[0], np.cumsum(KHW)]).astype(int)        # kh col offsets


def _f32r(ap):
    return ap.bitcast(F32R)


def _build_nc():
    nc = bacc.Bacc("TRN2", target_bir_lowering=False, debug=False,
                   num_devices=NCORES)

    din = {}
    def inp(name, shape, dt=F32):
        din[name] = nc.dram_tensor(name, list(shape), dt, kind="ExternalInput").ap()
        return din[name]

    hsT = inp("hsT", (HID, S), F32R)
    wq = inp("wq", (HID, 128), F32R)
    wk = inp("wk", (HID, 128), F32R)
    wv = inp("wv", (HID, 128), F32R)
    wob = inp("wob", (128, S), BF16)
    w1p = inp("w1p", (128, 4 * 128), BF16)       # packed per-head W1 tiles
    w2p = inp("w2p", (128, 128), BF16)           # packed per-head W2 tiles
    ropetab = inp("ropetab", (128, 4 * S))       # [tcq|tsq|tck|tsk]
    zrd = inp("zrd", (128, 2 * S))               # [zrep|rden]
    consts = inp("consts", (128, 2))             # [eps, row0big]
    pmat = inp("pmat", (128, 128), F32R)
    kh = inp("kh", (128, int(KHOFF[-1])))        # one-hot k_i-1 per chunk
    jbias = inp("jbias", (128, S), FP16)         # j - (S-1)
    repm = inp("repm", (128, 16 * 128), FP16)    # REP one-hot (b-major, h)

    outp = nc.dram_tensor("outp", [S, S], F32, kind="ExternalOutput").ap()

    with tile.TileContext(nc) as tc, ExitStack() as ctx:
        # ---------------- persistent pools ----------------
        pers = ctx.enter_context(tc.tile_pool(name="pers", bufs=1))
        qTr = pers.tile([128, S], F32R, tag="qTr")
        kTr = pers.tile([128, S], F32R, tag="kTr")
        vT = pers.tile([128, S], F32, tag="vT")
        vTg = pers.tile([128, S], F32, tag="vTg")
        epsv = pers.tile([128, S], F32, tag="epsv")
        zrdt = pers.tile([128, 2 * S], F32, tag="zrdt")
        zr = zrdt[:, 0:S]
        rd = zrdt[:, S:2 * S]
        comb_sum = pers.tile([128, S], BF16, tag="comb_sum")
        comb_mean = pers.tile([128, S], BF16, tag="comb_mean")
        comb_mx = pers.tile([128, S], BF16, tag="comb_mx")
        comb_var = pers.tile([128, S], BF16, tag="comb_var")
        h1sb = [pers.tile([128, S], BF16, tag=f"h1sb{h}", name=f"h1sb{h}")
                for h in range(2)]
        houtT = pers.tile([128, S], BF16, tag="houtT")
        identb = pers.tile([128, 128], BF16, tag="identb")
        khs = pers.tile([128, int(KHOFF[-1])], F32, tag="khs")
        jbs = pers.tile([128, S], FP16, tag="jbs")
        reps = pers.tile([128, 16 * 128], FP16, tag="reps")
        vAll = [pers.tile([128, 256], BF16, tag=f"vAll{jb}", name=f"vAll{jb}")
                for jb in range(NCHUNK)]
        adjT = [[pers.tile([128, 128 * (NCHUNK - jb)], BF16,
                           tag=f"adjT{h}_{jb}", name=f"adjT{h}_{jb}")
                 for jb in range(NCHUNK)] for h in range(2)]
        wot = pers.tile([128, S], BF16, tag="wot")
        w1s = pers.tile([128, 4 * 128], BF16, tag="w1s")
        w2s = pers.tile([128, 128], BF16, tag="w2s")

        make_identity(nc, identb[:])
        nc.sync.dma_start(zrdt[:], zrd)
        nc.sync.dma_start(khs[:], kh)
        nc.sync.dma_start(jbs[:], jbias)
        nc.sync.dma_start(reps[:], repm)
        nc.sync.dma_start(wot[:], wob)
        nc.sync.dma_start(w1s[:], w1p)
        nc.sync.dma_start(w2s[:], w2p)

        cst = pers.tile([128, 2], F32, tag="cst")
        nc.sync.dma_start(cst[:], consts)

        # ---------------- phase A: projections + rope ----------------
        with ExitStack() as actx:
            apool = actx.enter_context(tc.tile_pool(name="aw", bufs=1))
            hspool = actx.enter_context(tc.tile_pool(name="hs", bufs=2))
            apsum = actx.enter_context(
                tc.tile_pool(name="apsum", bufs=1, space="PSUM"))

            ropet = apool.tile([128, 4 * S], F32, tag="ropet")
            nc.sync.dma_start(ropet[:], ropetab)
            tq = ropet[:, 0:S]
            tsq_t = ropet[:, S:2 * S]
            tk = ropet[:, 2 * S:3 * S]
            tsk_t = ropet[:, 3 * S:4 * S]

            # one DMA per weight: [128, k, 128] <- rows 128k..128k+127
            wqt = apool.tile([128, 8 * 128], F32R, tag="wqt")
            wkt = apool.tile([128, 8 * 128], F32R, tag="wkt")
            wvt = apool.tile([128, 8 * 128], F32R, tag="wvt")
            for t, w in ((wqt, wq), (wkt, wk), (wvt, wv)):
                nc.sync.dma_start(
                    t[:].rearrange("p (k c) -> p k c", k=8),
                    w.rearrange("(k p) c -> p k c", k=8))

            qps = apsum.tile([128, S], F32, tag="qps")
            kps = apsum.tile([128, S], F32, tag="kps")
            vps = apsum.tile([128, S], F32, tag="vps")
            for k in range(8):
                hst = hspool.tile([128, S], F32R, tag="hst")
                nc.sync.dma_start(hst[:], hsT[128 * k:128 * (k + 1), :])
                for n in range(2):
                    sl = slice(512 * n, 512 * (n + 1))
                    ksl = slice(128 * k, 128 * (k + 1))
                    nc.tensor.matmul(qps[:, sl], lhsT=wqt[:, ksl],
                                     rhs=hst[:, sl],
                                     start=(k == 0), stop=(k == 7))
                    nc.tensor.matmul(kps[:, sl], lhsT=wkt[:, ksl],
                                     rhs=hst[:, sl],
                                     start=(k == 0), stop=(k == 7))
                    nc.tensor.matmul(vps[:, sl], lhsT=wvt[:, ksl],
                                     rhs=hst[:, sl],
                                     start=(k == 0), stop=(k == 7))

            # rope: out = x*C + (PM @ x)*Sn where PM is the signed rotate-half
            # permutation (exact on PE).
            pmt = apool.tile([128, 128], F32R, tag="pmt")
            nc.sync.dma_start(pmt[:], pmat)

            def rope(dst, src_ps, ctab, stab):
                xsb = hspool.tile([128, S], F32R, tag="ropex")
                nc.scalar.copy(xsb[:], src_ps[:])
                rot = hspool.tile([128, S], F32, tag="roper")
                for n in range(2):
                    sl = slice(512 * n, 512 * (n + 1))
                    rps = apsum.tile([128, 512], F32, tag="ropeps")
                    nc.tensor.matmul(rps[:], lhsT=pmt[:],
                                     rhs=xsb[:, sl],
                                     start=True, stop=True)
                    nc.scalar.copy(rot[:, sl], rps[:])
                nc.vector.tensor_tensor(dst[:], xsb[:], ctab, op=ALU.mult)
                nc.vector.tensor_tensor(rot[:], rot[:], stab, op=ALU.mult)
                nc.vector.tensor_tensor(dst[:], dst[:], rot[:], op=ALU.add)

            rope(qTr, qps, tq, tsq_t)
            rope(kTr, kps, tk, tsk_t)

            nc.scalar.copy(vT[:], vps[:])
            nc.scalar.copy(vTg[:], vps[:])
            nc.vector.memset(vTg[:, S - 1:S], NEG)
            nc.vector.tensor_scalar(epsv[:], vT[:], cst[:, 0:1], None,
                                    op0=ALU.mult)

        # v_all blocks: bf16 copies of v and v^2, PE-transposed per block.
        with ExitStack() as vctx:
            vtp = vctx.enter_context(tc.tile_pool(name="vtt", bufs=1))
            vpsum = vctx.enter_context(
                tc.tile_pool(name="vtp", bufs=2, space="PSUM"))
            vTb = vtp.tile([128, S], BF16, tag="vTb")
            vTsq = vtp.tile([128, S], BF16, tag="vTsq")
            nc.vector.tensor_copy(vTb[:], vT[:])
            nc.scalar.activation(vTsq[:], vT[:], AF.Square)
            for jb in range(NCHUNK):
                sl = slice(128 * jb, 128 * (jb + 1))
                tp = vpsum.tile([128, 256], BF16, tag="vtp")
                nc.tensor.transpose(tp[:, 0:128], vTb[:, sl], identb[:])
                nc.tensor.transpose(tp[:, 128:256], vTsq[:, sl], identb[:])
                # vAll layout per head h: [v_h (64) | v^2_h (64)]
                for h in range(2):
                    nc.scalar.copy(vAll[jb][:, 128 * h:128 * h + 64],
                                   tp[:, 64 * h:64 * h + 64])
                    nc.scalar.copy(vAll[jb][:, 128 * h + 64:128 * h + 128],
                                   tp[:, 128 + 64 * h:128 + 64 * h + 64])

        # ---------------- phase B: scores / top-k / adjacency ----------------
        scpsum = ctx.enter_context(tc.tile_pool(name="scps", bufs=2, space="PSUM"))
        mpsum = ctx.enter_context(tc.tile_pool(name="mps", bufs=3, space="PSUM"))
        ipsum = ctx.enter_context(tc.tile_pool(name="ips", bufs=1, space="PSUM"))
        gpool = ctx.enter_context(tc.tile_pool(name="gp", bufs=3))
        tkpool = ctx.enter_context(tc.tile_pool(name="tkp", bufs=3))
        idxp = ctx.enter_context(tc.tile_pool(name="idxp", bufs=3))
        gatp = ctx.enter_context(tc.tile_pool(name="gatp", bufs=3))

        for c in range(NCHUNK):
            W = 128 * (c + 1)
            R = RC[c]
            kp = KPAD[c]
            idxl = {}
            for h in range(2):
                po = 64 * h
                sc = scpsum.tile([128, W], F32, tag="sc")
                for n0 in range(0, W, 512):
                    n1 = min(n0 + 512, W)
                    nc.tensor.matmul(
                        sc[:, n0:n1],
                        lhsT=qTr[po:po + 64, 128 * c:128 * (c + 1)],
                        rhs=kTr[po:po + 64, n0:n1], start=True, stop=True)

                msk = gpool.tile([128, W], U8, tag="msk")
                nc.vector.tensor_scalar(msk[:], sc[:], float(THR), None,
                                        op0=ALU.is_ge)
                g = gpool.tile([128, W], F32, tag="g")
                nc.vector.select(g[:], msk[:], sc[:], zr[:, 0:W])
                nc.gpsimd.affine_select(
                    out=g[:, 128 * c:W], in_=g[:, 128 * c:W],
                    compare_op=ALU.is_gt, fill=float(NEG),
                    base=0, pattern=[[-1, 128]], channel_multiplier=1)

                gw = gpool.tile([128, W], F32, tag="gw")
                nc.vector.tensor_copy(gw[:], g[:])

                vals = tkpool.tile([128, 8 * R], F32, tag="vals")
                for r in range(R):
                    sl = slice(8 * r, 8 * r + 8)
                    nc.vector.max(vals[:, sl], gw[:])
                    nc.vector.match_replace(gw[:], vals[:, sl], gw[:],
                                            float(NEG))

                # cutoff = vals[i, k_i-1] via one-hot mask + max-reduce
                tmp = tkpool.tile([128, 8 * R], F32, tag="ctmp")
                nc.vector.tensor_tensor(
                    tmp[:], vals[:],
                    khs[:, int(KHOFF[c]):int(KHOFF[c]) + 8 * R], op=ALU.mult)
                cut = tkpool.tile([128, 1], F32, tag="cut")
                nc.vector.tensor_reduce(cut[:], tmp[:], axis=mybir.AxisListType.X,
                                        op=ALU.max)
                if c == 0:
                    # row 0 selects nothing: push cutoff to +inf
                    nc.vector.tensor_tensor(cut[:], cut[:], cst[:, 1:2],
                                            op=ALU.add)

                adj = gpool.tile([128, W], BF16, tag="adj")
                nc.vector.tensor_scalar(adj[:], g[:], cut[:, 0:1], None,
                                        op0=ALU.is_ge)

                # transposed adjacency blocks for the PE aggregation
                for jb0 in range(0, c + 1, 4):
                    jb1 = min(jb0 + 4, c + 1)
                    tp = mpsum.tile([128, 128 * (jb1 - jb0)], BF16, tag="ps1")
                    for jb in range(jb0, jb1):
                        nc.tensor.transpose(
                            tp[:, 128 * (jb - jb0):128 * (jb - jb0) + 128],
                            adj[:, 128 * jb:128 * (jb + 1)], identb[:])
                    for jb in range(jb0, jb1):
                        nc.scalar.copy(
                            adjT[h][jb][:, 128 * (c - jb):128 * (c - jb) + 128],
                            tp[:, 128 * (jb - jb0):128 * (jb - jb0) + 128])

                # ranks: prefix-sum of adj (Pool), then masked scatter targets
                ranks = idxp.tile([128, W], I16, tag="ranks")
                nc.gpsimd.tensor_tensor_scan(
                    ranks[:], adj[:], adj[:], -1.0,
                    op0=ALU.add, op1=ALU.bypass)
                imask = idxp.tile([128, W], I16, tag="imask")
                nc.vector.memset(imask[:], -1)
                nc.vector.copy_predicated(imask[:], adj[:], ranks[:])

                # compact per-row selected column ids (as j-(S-1), fp16)
                il = idxp.tile([128, kp], FP16, tag=f"il{h}")
                nc.gpsimd.local_scatter(
                    il[:], jbs[:, 0:W], imask[:],
                    channels=128, num_elems=kp, num_idxs=W)
                idxl[h] = il

            # replicate idx lists into all 8 gpsimd groups (fp16 matmuls),
            # convert to int16 with +S-1 sentinel bias, gather + max-reduce.
            for b0 in range(0, 8, 4):
                ips = ipsum.tile([128, 4 * kp], F32, tag="ips")
                for b in range(b0, b0 + 4):
                    osl = slice((b - b0) * kp, (b - b0 + 1) * kp)
                    nc.tensor.matmul(
                        ips[:, osl], lhsT=reps[:, 128 * (2 * b):128 * (2 * b) + 128],
                        rhs=idxl[0][:], start=True, stop=False)
                    nc.tensor.matmul(
                        ips[:, osl], lhsT=reps[:, 128 * (2 * b + 1):128 * (2 * b + 2)],
                        rhs=idxl[1][:], start=False, stop=True)
                irep = gatp.tile([128, 4 * kp], I16, tag="irep")
                nc.scalar.activation(irep[:], ips[:], AF.Copy,
                                     bias=float(S - 1), scale=1.0)
                for b in range(b0, b0 + 4):
                    gat = gatp.tile([128, 16 * kp], F32, tag="gat")
                    nc.gpsimd.ap_gather(
                        gat[:], vTg[:], irep[:, (b - b0) * kp:(b - b0 + 1) * kp],
                        channels=128, num_elems=S, d=1, num_idxs=16 * kp)
                    nc.vector.tensor_reduce(
                        comb_mx[:, 128 * c + 16 * b:128 * c + 16 * b + 16],
                        gat[:].rearrange("p (s q) -> p q s", q=16),
                        axis=mybir.AxisListType.X, op=ALU.max)

        # row 0 selects nothing -> mx must be 0
        nc.vector.memset(comb_mx[:, 0:1], 0.0)

        # ---------------- phase C: aggregation + moments ----------------
        tmpp = ctx.enter_context(tc.tile_pool(name="tmpp", bufs=2))
        for h in range(2):
            po = 64 * h
            for c in range(NCHUNK):
                cc = slice(128 * c, 128 * (c + 1))
                pa = mpsum.tile([128, 128], F32, tag="ps1")
                for jb in range(c + 1):
                    lhs = vAll[jb][:, 128 * h:128 * (h + 1)]
                    nc.tensor.matmul(
                        pa[:], lhsT=lhs,
                        rhs=adjT[h][jb][:, 128 * (c - jb):128 * (c - jb) + 128],
                        start=(jb == 0), stop=(jb == c))
                nc.scalar.copy(comb_sum[po:po + 64, cc], pa[0:64, :])
                nc.vector.tensor_tensor(comb_mean[po:po + 64, cc], pa[0:64, :],
                                        rd[po:po + 64, cc], op=ALU.mult)
                nc.vector.tensor_tensor(comb_var[po:po + 64, cc], pa[64:128, :],
                                        rd[po:po + 64, cc], op=ALU.mult)
                sq = tmpp.tile([128, 128], BF16, tag="sq")
                nc.vector.tensor_tensor(sq[po:po + 64, :],
                                        comb_mean[po:po + 64, cc],
                                        comb_mean[po:po + 64, cc], op=ALU.mult)
                nc.vector.tensor_tensor(comb_var[po:po + 64, cc],
                                        comb_var[po:po + 64, cc],
                                        sq[po:po + 64, :], op=ALU.subtract)
                nc.vector.tensor_scalar(comb_var[po:po + 64, cc],
                                        comb_var[po:po + 64, cc], 0.0, None,
                                        op0=ALU.max)

        # ---------------- phase D: GIN MLP + residual ----------------
        for h in range(2):
            po = 64 * h
            combs = [comb_sum, comb_mean, comb_mx, comb_var]
            for n in range(2):
                sl = slice(512 * n, 512 * (n + 1))
                h1p = mpsum.tile([128, 512], F32, tag="ps1")
                for x in range(4):
                    nc.tensor.matmul(h1p[:], lhsT=w1s[po:po + 64,
                                                     128 * x:128 * (x + 1)],
                                     rhs=combs[x][po:po + 64, sl],
                                     start=(x == 0), stop=(x == 3))
                sg = tmpp.tile([128, 512], BF16, tag="sg")
                nc.scalar.activation(sg[:], h1p[:], AF.Sigmoid)
                nc.vector.tensor_tensor(h1sb[h][:, sl], h1p[:], sg[:],
                                        op=ALU.mult)
                hop = mpsum.tile([64, 512], F32, tag="ps1")
                nc.tensor.matmul(hop[:], lhsT=w2s[:, 64 * h:64 * (h + 1)],
                                 rhs=h1sb[h][:, sl], start=True, stop=True)
                nc.vector.tensor_tensor(houtT[po:po + 64, sl], hop[:],
                                        epsv[po:po + 64, sl], op=ALU.add)

        # ---------------- phase E: o_proj partial ----------------
        opool = ctx.enter_context(tc.tile_pool(name="op", bufs=2))
        for c in range(NCHUNK):
            osb = opool.tile([128, S], F32, tag="osb")
            for n in range(2):
                sl = slice(512 * n, 512 * (n + 1))
                op = mpsum.tile([128, 512], F32, tag="ps1")
                nc.tensor.matmul(op[:], lhsT=houtT[:, 128 * c:128 * (c + 1)],
                                 rhs=wot[:, sl], start=True, stop=True)
                nc.scalar.copy(osb[:, sl], op[:])
            nc.sync.dma_start(outp[128 * c:128 * (c + 1), :], osb[:])

    nc.compile()
    return nc


def _host_inputs(inputs):
    """Build the 8 per-core input dicts from the full problem inputs."""
    hs = np.ascontiguousarray(np.asarray(inputs["hidden_states"],
                                         dtype=np.float32)[0])      # (S, HID)
    Wq = np.asarray(inputs["Wq"], dtype=np.float32)
    Wk = np.asarray(inputs["Wk"], dtype=np.float32)
    Wv = np.asarray(inputs["Wv"], dtype=np.float32)
    Wo = np.asarray(inputs["Wo"], dtype=np.float32)
    W1 = np.asarray(inputs["W1"], dtype=np.float32)
    W2 = np.asarray(inputs["W2"], dtype=np.float32)
    eps = np.float32(np.asarray(inputs["eps"]).reshape(-1)[0])
    pos = np.asarray(inputs["position_ids"]).reshape(-1).astype(np.float32)

    hsT = np.ascontiguousarray(hs.T)

    inv = (1.0 / (np.float32(BASE) **
                  (np.arange(0, D, 2, dtype=np.float32) / np.float32(D))))
    ang = pos[:, None] * inv[None, :].astype(np.float32)            # (S, 32)
    c32 = np.cos(ang).astype(np.float32).T                          # (32, S)
    s32 = np.sin(ang).astype(np.float32).T
    stack = lambda a: np.concatenate([a, a, a, a], axis=0)          # (128, S)
    ropetab = np.concatenate([
        stack((c32 / np.float32(8.0)).astype(np.float32)),
        stack((s32 / np.float32(8.0)).astype(np.float32)),
        stack(c32), stack(s32)], axis=1)                            # (128, 4S)

    j = np.arange(S, dtype=np.float32)
    zrow = (np.float32(DELTA) * (np.float32(S) - j)).astype(np.float32)
    denom = np.maximum(KV, 1).astype(np.float32)
    zrd = np.concatenate([
        np.broadcast_to(zrow, (128, S)),
        np.broadcast_to((np.float32(1.0) / denom), (128, S))],
        axis=1).astype(np.float32).copy()

    consts = np.zeros((128, 2), dtype=np.float32)
    consts[:, 0] = eps
    consts[0, 1] = np.float32(1e30)

    pmat = np.zeros((128, 128), dtype=np.float32)
    for h in range(2):
        b = 64 * h
        for r in range(32):
            pmat[b + 32 + r, b + r] = -1.0      # rot[lo] = -x[hi]
            pmat[b + r, b + 32 + r] = 1.0       # rot[hi] = +x[lo]

    # one-hot cutoff position k_i-1 per chunk (row 0 stays all-zero)
    kh = np.zeros((128, int(KHOFF[-1])), dtype=np.float32)
    for c in range(NCHUNK):
        krow = KV[128 * c:128 * (c + 1)]
        for p in range(128):
            if krow[p] > 0:
                kh[p, int(KHOFF[c]) + int(krow[p]) - 1] = 1.0

    jbias = np.broadcast_to((j - np.float32(S - 1)).astype(np.float16),
                            (128, S)).copy()

    # REP matrices: out partition p takes idx row 16b + (p%16) of head h(p)
    repm = np.zeros((128, 16 * 128), dtype=np.float16)
    for b in range(8):
        for h in range(2):
            m = np.zeros((128, 128), dtype=np.float16)   # [contract c, part p]
            prange = range(64 * h, 64 * h + 64)
            for p in prange:
                m[16 * b + (p % 16), p] = 1.0
            repm[:, 128 * (2 * b + h):128 * (2 * b + h + 1)] = m

    # packed per-head MLP weights (bf16 stored as uint16 view -> use f32->bf16)
    import ml_dtypes
    w1pk = np.zeros((128, 4 * 128), dtype=ml_dtypes.bfloat16)
    w2pk = np.zeros((128, 128), dtype=ml_dtypes.bfloat16)

    maps = []
    for core in range(NCORES):
        h0 = 2 * core
        sl = slice(h0 * D, (h0 + 2) * D)
        w1pk_c = w1pk.copy()
        for h in range(2):
            for x in range(4):
                w1pk_c[64 * h:64 * h + 64, 128 * x:128 * (x + 1)] = \
                    W1[h0 + h, 64 * x:64 * (x + 1), :].astype(ml_dtypes.bfloat16)
        w2pk_c = w2pk.copy()
        for h in range(2):
            w2pk_c[:, 64 * h:64 * h + 64] = \
                W2[h0 + h].astype(ml_dtypes.bfloat16)
        maps.append({
            "hsT": hsT,
            "wq": np.ascontiguousarray(Wq[:, sl]),
            "wk": np.ascontiguousarray(Wk[:, sl]),
            "wv": np.ascontiguousarray(Wv[:, sl]),
            "wob": np.ascontiguousarray(Wo[sl, :]).astype(ml_dtypes.bfloat16),
            "w1p": w1pk_c,
            "w2p": w2pk_c,
            "ropetab": ropetab, "zrd": zrd, "consts": consts, "pmat": pmat,
            "kh": kh, "jbias": jbias, "repm": repm,
        })
    return maps


_NC_CACHE = {}


def _get_nc():
    if "nc" not in _NC_CACHE:
        _NC_CACHE["nc"] = _build_nc()
    return _NC_CACHE["nc"]


def _get_runner():
    """Compile once; return (fn, in_names, zero_outs, mesh/sharding)."""
    if "runner" in _NC_CACHE:
        return _NC_CACHE["runner"]
    import jax
    from jax.sharding import Mesh, PartitionSpec, NamedSharding
    from jax.experimental.shard_map import shard_map
    from concourse import bass2jax

    nc = _get_nc()
    bass2jax.install_neuronx_cc_hook()
    partition_name = (nc.partition_id_tensor.name
                      if nc.partition_id_tensor else None)
    in_names, out_names, out_avals, zero_outs = [], [], [], []
    for alloc in nc.m.functions[0].allocations:
        if not isinstance(alloc, mybir.MemoryLocationSet):
            continue
        name = alloc.memorylocations[0].name
        if alloc.kind == "ExternalInput":
            if name != partition_name:
                in_names.append(name)
        elif alloc.kind == "ExternalOutput":
            out_names.append(name)
            shape = tuple(alloc.tensor_shape)
            dtype = mybir.dt.np(alloc.dtype)
            out_avals.append(jax.core.ShapedArray(shape, dtype))
            zero_outs.append(np.zeros(shape, dtype))
    all_in = in_names + out_names + ([partition_name] if partition_name else [])

    def _body(*args):
        ops = list(args)
        if partition_name:
            ops.append(bass2jax.partition_id_tensor())
        return tuple(bass2jax._bass_exec_p.bind(
            *ops, out_avals=tuple(out_avals), in_names=tuple(all_in),
            out_names=tuple(out_names), lowering_input_output_aliases=(),
            sim_require_finite=True, sim_require_nnan=True, nc=nc))

    devices = jax.devices()[:NCORES]
    mesh = Mesh(np.asarray(devices), ("core",))
    spec = PartitionSpec("core")
    fn = jax.jit(shard_map(
        _body, mesh=mesh,
        in_specs=(spec,) * (len(in_names) + len(out_names)),
        out_specs=(spec,) * len(out_names), check_rep=False))
    sh = NamedSharding(mesh, spec)
    zo_dev = [jax.device_put(np.concatenate([zo] * NCORES, axis=0), sh)
              for zo in zero_outs]
    _NC_CACHE["runner"] = (fn, in_names, zo_dev, sh, jax)
    return _NC_CACHE["runner"]


def kernel(**inputs) -> np.ndarray:
    fn, in_names, zo_dev, sh, jax = _get_runner()
    maps = _host_inputs(inputs)
    args = []
    for name in in_names:
        ci = np.concatenate([np.asarray(maps[c][name]) for c in range(NCORES)],
                            axis=0)
        args.append(jax.device_put(ci, sh))
    args.extend(zo_dev)
    outs = fn(*args)
    full = np.asarray(outs[0])                    # (NCORES*S, S) concat
    out = full.reshape(NCORES, S, S).sum(axis=0, dtype=np.float32)
    return out[None].astype(np.float32)
